# revision 40
# baseline (speedup 1.0000x reference)
"""Trainium2 Bass kernel for the Kalman graphical-model message-passing problem.

reference math (B=64, D=8, M=4, S=50000):
    m1 = -Qinv @ (xs - F @ x_past)            (B, D, S)
    m2 = FtQinv @ (x_fut - F @ xs)            (B, D, S)
    m3 = HtRinv @ ys_t - (HtRinv @ H) @ xs    (B, D, S)
with x_past/x_fut edge-replicated 1-sample shifts of xs along S.

Design (everything bf16 on the wire; the rel-err gate is 2e-2 and bf16
end-to-end measures ~7.5e-3):

  * Algebra: six host-precomputed 8x8 (or 8x4) weight matrices turn every
    output into plain accumulating matmuls over shifted views of one tile:
      m1 = A1 x_t + B1 x_{t-1}     A1 = -Qinv,     B1 = Qinv F
      m2 = A2 x_t + B2 x_{t+1}     A2 = -F'QinvF,  B2 = F'Qinv
      m3 = A3 x_t + C3 y_t         C3 = H'Rinv,    A3 = -(C3 H)
    => 6 matmuls per 512-column chunk (the m2_t = -F' m1_{t+1} chaining
    variant needs only 5 but serializes PE behind the m1 PSUM drain and
    measured slower).

  * Layout: per batch one supertile of NG=16 groups x gw=s/16 samples.
    Partition 8g+j = (group g, state j); columns = samples with one halo
    column on each side, so cur/past/fut are column offsets 1/0/2 of the
    same tile.  The host PRE-PACKS xs into this exact SBUF image (edge
    replication via clipped gather -> no halo DMAs, no edge cases on
    device), and ys transposed into partition 4g+m so the ys contraction
    over m is a single 64-partition matmul per chunk (vs 4 stride-4
    matmuls).  Weights are 16x block-diagonal [128, 128] lhsT images.

  * Per batch: 2 load DMAs (x ~800KB, y ~400KB) + 6 store DMAs, all with
    >=2KB contiguous runs.  Per-core traffic ~28.8MB (~2x below fp32).
    Loads issue on the ACT HWDGE ring; stores are spread over the SP HWDGE
    ring AND the gpsimd SWDGE ring (o3 alternates per batch to balance
    bytes) -- each SDMA engine round-robins between rings at packet
    granularity, so the extra ring measurably lifts aggregate HBM
    throughput (~250 -> ~315 GB/s).  Stores go out in two column segments
    so segment A streams while the tail chunks still compute.

  * PSUM fp32 (3 pools: 3+2+3 banks), outputs cast to bf16 on the
    PSUM->SBUF copy; m1/m2 casts on DVE, m3 on ACT to split the copy load.
"""

import os
from contextlib import ExitStack

import ml_dtypes
import numpy as np

import concourse.bacc as bacc
import concourse.bass as bass
import concourse.mybir as mybir
import concourse.tile as tile
from concourse.bass_utils import run_bass_kernel_spmd

F32 = mybir.dt.float32
BF16 = mybir.dt.bfloat16
NPBF16 = ml_dtypes.bfloat16

B, D, M, S = 64, 8, 4, 50000
N_CORES = 8
BC = B // N_CORES  # batches per core
NG = 16            # sample groups packed into the 128 partitions
MW = 512           # matmul free-dim / PSUM bank width


def _geom(s):
    assert s % NG == 0, s
    gw = s // NG   # samples per group
    xc = gw + 2    # x cols: 1 past halo + gw + 1 fut halo (cols 0..gw+1 used)
    xc += xc % 2   # pad to even row bytes
    yc = gw + (gw % 2)
    return gw, xc, yc


def _build_nc(bc=BC, s=S):
    variant = os.environ.get("KERNEL_VARIANT", "full")  # perf bisection only
    gw, xc, yc = _geom(s)

    nc = bacc.Bacc(trn_type="TRN2")
    xp = nc.dram_tensor("xp", [bc, 128, xc], BF16, kind="ExternalInput")
    yp = nc.dram_tensor("yp", [bc, 64, yc], BF16, kind="ExternalInput")
    w = nc.dram_tensor("w_all", [128, 7 * 128], BF16, kind="ExternalInput")
    m_all = nc.dram_tensor("m_all", [bc, D, 3, s], BF16, kind="ExternalOutput")

    with tile.TileContext(nc) as tc, ExitStack() as ctx:
        singles = ctx.enter_context(tc.tile_pool(name="singles", bufs=1))
        xpool = ctx.enter_context(tc.tile_pool(name="xp", bufs=8))
        ypool = ctx.enter_context(tc.tile_pool(name="yp", bufs=8))
        o1p = ctx.enter_context(tc.tile_pool(name="o1", bufs=4))
        o2p = ctx.enter_context(tc.tile_pool(name="o2", bufs=4))
        o3p = ctx.enter_context(tc.tile_pool(name="o3", bufs=4))
        pp1 = ctx.enter_context(tc.tile_pool(name="pp1", bufs=3, space="PSUM"))
        pp2 = ctx.enter_context(tc.tile_pool(name="pp2", bufs=2, space="PSUM"))
        pp3 = ctx.enter_context(tc.tile_pool(name="pp3", bufs=3, space="PSUM"))

        w_sb = singles.tile([128, 7 * 128], BF16, tag="w")
        nc.sync.dma_start(out=w_sb[:], in_=w[:, :])

        wA1 = w_sb[:, 0:128]
        wB1 = w_sb[:, 128:256]
        wW2 = w_sb[:, 256:384]
        wA2 = w_sb[:, 384:512]
        wB2 = w_sb[:, 512:640]
        wA3 = w_sb[:, 640:768]
        wC3 = w_sb[0:64, 768:896]

        for b in range(bc):
            xoff = b * 128 * xc
            yoff = b * 64 * yc
            ooff = b * D * 3 * s

            x_t = xpool.tile([128, xc], BF16, tag="x")
            nc.scalar.dma_start(out=x_t[:], in_=bass.AP(xp, xoff, [[xc, 128], [1, xc]]))
            y_t = ypool.tile([64, yc], BF16, tag="y")
            nc.scalar.dma_start(out=y_t[:], in_=bass.AP(yp, yoff, [[yc, 64], [1, yc]]))
            if variant == "loads":
                continue

            o1 = o1p.tile([128, gw], BF16, tag="o1", name=f"o1_{b}")
            o2 = o2p.tile([128, gw], BF16, tag="o2", name=f"o2_{b}")
            o3 = o3p.tile([128, gw], BF16, tag="o3", name=f"o3_{b}")

            for h0 in range(0, gw, MW):
                hw = min(MW, gw - h0)
                cur = x_t[:, 1 + h0 : 1 + h0 + hw]
                past = x_t[:, h0 : h0 + hw]
                fut = x_t[:, 2 + h0 : 2 + h0 + hw]
                p1 = pp1.tile([128, MW], F32, tag="p1", name=f"p1_{b}_{h0}")
                nc.tensor.matmul(p1[:, 0:hw], wA1, cur, start=True, stop=False)
                nc.tensor.matmul(p1[:, 0:hw], wB1, past, start=False, stop=True)
                nc.vector.tensor_copy(out=o1[:, h0 : h0 + hw], in_=p1[:, 0:hw])
                p3 = pp3.tile([128, MW], F32, tag="p3", name=f"p3_{b}_{h0}")
                nc.tensor.matmul(p3[:, 0:hw], wA3, cur, start=True, stop=False)
                nc.tensor.matmul(
                    p3[:, 0:hw], wC3, y_t[:, h0 : h0 + hw], start=False, stop=True
                )
                nc.scalar.copy(out=o3[:, h0 : h0 + hw], in_=p3[:, 0:hw])
                p2 = pp2.tile([128, MW], F32, tag="p2", name=f"p2_{b}_{h0}")
                nc.tensor.matmul(p2[:, 0:hw], wA2, cur, start=True, stop=False)
                nc.tensor.matmul(p2[:, 0:hw], wB2, fut, start=False, stop=True)
                nc.vector.tensor_copy(out=o2[:, h0 : h0 + hw], in_=p2[:, 0:hw])

            if variant == "nostores":
                continue
            # stores split across the sync (HWDGE) and gpsimd (SWDGE) rings:
            # each SDMA engine round-robins between rings at packet
            # granularity, so a third stream hides more HBM latency.  o3
            # alternates so both store rings carry ~equal bytes; two column
            # segments let segment A stream while tail chunks compute.
            st_split = min(4 * MW, gw)
            for o_idx, o_t in ((0, o1), (1, o2), (2, o3)):
                if o_idx == 0:
                    eng = nc.sync
                elif o_idx == 1:
                    eng = nc.gpsimd
                else:
                    eng = nc.sync if b % 2 else nc.gpsimd
                eng.dma_start(
                    out=bass.AP(
                        m_all, ooff + o_idx * s, [[gw, NG], [3 * s, D], [1, st_split]]
                    ),
                    in_=o_t[:, 0:st_split],
                )
                if st_split < gw:
                    eng.dma_start(
                        out=bass.AP(
                            m_all,
                            ooff + o_idx * s + st_split,
                            [[gw, NG], [3 * s, D], [1, gw - st_split]],
                        ),
                        in_=o_t[:, st_split:gw],
                    )
    nc.finalize()
    return nc


def _build_weights(F, H, Q, R):
    """Host-side precompute (init-time work in the torch module)."""
    F64 = np.asarray(F, np.float64)
    H64 = np.asarray(H, np.float64)
    Qinv = np.linalg.inv(np.asarray(Q, np.float64))
    Rinv = np.linalg.inv(np.asarray(R, np.float64))
    A1 = -Qinv
    B1 = Qinv @ F64
    W2 = -F64.T
    C3 = H64.T @ Rinv          # (D, M)
    A3 = -(C3 @ H64)

    A2 = -(F64.T @ Qinv @ F64)
    B2 = F64.T @ Qinv

    eye = np.eye(NG)
    w = np.zeros((128, 7 * 128), NPBF16)
    for i, A in enumerate([A1, B1, W2, A2, B2, A3]):
        # lhsT[8g+j, 8g+i] = A[i, j]  ->  block-diag of A.T
        w[:, i * 128 : (i + 1) * 128] = np.kron(eye, A.T).astype(NPBF16)
    w[0:64, 768:896] = np.kron(eye, C3.T).astype(NPBF16)  # [4g+m, 8g+i] = C3[i, m]
    return w


def _pack_inputs(xs, ys, s):
    """xs (nb, D, s), ys (nb, s, M) f32 -> device images (bf16).

    xp[b, 8g+j, c] = xs[b, j, clip(g*gw + c - 1)]   (c in [0, xc))
    yp[b, 4g+m, c] = ys[b, clip(g*gw + c), m]       (c in [0, yc))
    """
    gw, xc, yc = _geom(s)
    nb = xs.shape[0]
    xs_bf = np.asarray(xs, np.float32).astype(NPBF16)
    g = np.arange(NG)[:, None] * gw
    xcols = np.clip(g + np.arange(xc)[None, :] - 1, 0, s - 1)  # (NG, xc)
    xp = xs_bf[:, :, xcols]                      # (nb, D, NG, xc)
    xp = np.ascontiguousarray(np.swapaxes(xp, 1, 2)).reshape(nb, 128, xc)

    ys_bf = np.swapaxes(np.asarray(ys, np.float32).astype(NPBF16), 1, 2)  # (nb, M, s)
    ycols = np.clip(g + np.arange(yc)[None, :], 0, s - 1)      # (NG, yc)
    yp = ys_bf[:, :, ycols]                      # (nb, M, NG, yc)
    yp = np.ascontiguousarray(np.swapaxes(yp, 1, 2)).reshape(nb, 64, yc)
    return xp, yp


_CACHE = {}


def _get_nc(bc=BC, s=S):
    key = (bc, s)
    if key not in _CACHE:
        _CACHE[key] = _build_nc(bc, s)
    return _CACHE[key]


def run(xs, ys, F, H, Q, R, trace=False, bc=BC, s=S):
    """Shard across 8 cores, run, gather.  Returns ((m1, m2, m3), results)."""
    nb = xs.shape[0]
    assert nb == bc * N_CORES and xs.shape[1:] == (D, s), xs.shape
    assert ys.shape == (nb, s, M), ys.shape
    xp, yp = _pack_inputs(xs, ys, s)
    w_all = _build_weights(F, H, Q, R)

    nc = _get_nc(bc, s)
    in_maps = [
        {
            "xp": np.ascontiguousarray(xp[i * bc : (i + 1) * bc]),
            "yp": np.ascontiguousarray(yp[i * bc : (i + 1) * bc]),
            "w_all": w_all,
        }
        for i in range(N_CORES)
    ]
    res = run_bass_kernel_spmd(nc, in_maps, core_ids=list(range(N_CORES)), trace=trace)
    m_full = np.concatenate([r["m_all"] for r in res.results], axis=0)  # (B,D,3,s)
    outs = tuple(
        np.ascontiguousarray(m_full[:, :, i, :]).astype(np.float32) for i in range(3)
    )
    return outs, res


def kernel(xs, ys, F, H, Q, R):
    trace = bool(int(os.environ.get("KERNEL_TRACE", "0")))
    outs, _ = run(xs, ys, F, H, Q, R, trace=trace)
    return outs


# revision 41
# speedup vs baseline: 1.0558x; 1.0558x over previous
"""Trainium2 Bass kernel for the Kalman graphical-model message-passing problem.

reference math (B=64, D=8, M=4, S=50000):
    m1 = -Qinv @ (xs - F @ x_past)            (B, D, S)
    m2 = FtQinv @ (x_fut - F @ xs)            (B, D, S)
    m3 = HtRinv @ ys_t - (HtRinv @ H) @ xs    (B, D, S)
with x_past/x_fut edge-replicated 1-sample shifts of xs along S.

Design (everything bf16 on the wire; the rel-err gate is 2e-2 and bf16
end-to-end measures ~7.5e-3):

  * Algebra: six host-precomputed 8x8 (or 8x4) weight matrices turn every
    output into plain accumulating matmuls over shifted views of one tile:
      m1 = A1 x_t + B1 x_{t-1}     A1 = -Qinv,     B1 = Qinv F
      m2 = A2 x_t + B2 x_{t+1}     A2 = -F'QinvF,  B2 = F'Qinv
      m3 = A3 x_t + C3 y_t         C3 = H'Rinv,    A3 = -(C3 H)
    => 6 matmuls per 512-column chunk (the m2_t = -F' m1_{t+1} chaining
    variant needs only 5 but serializes PE behind the m1 PSUM drain and
    measured slower).

  * Layout: per batch one supertile of NG=16 groups x gw=s/16 samples.
    Partition 8g+j = (group g, state j); columns = samples with one halo
    column on each side, so cur/past/fut are column offsets 1/0/2 of the
    same tile.  The host PRE-PACKS xs into this exact SBUF image (edge
    replication via clipped gather -> no halo DMAs, no edge cases on
    device), and ys transposed into partition 4g+m so the ys contraction
    over m is a single 64-partition matmul per chunk (vs 4 stride-4
    matmuls).  Weights are 16x block-diagonal [128, 128] lhsT images.

  * Per batch: 2 load DMAs (x ~800KB, y ~400KB) + 6 store DMAs, all with
    >=2KB contiguous runs.  Per-core traffic ~28.8MB (~2x below fp32).
    Loads issue on the ACT HWDGE ring; stores are spread over the SP HWDGE
    ring AND the gpsimd SWDGE ring (o3 alternates per batch to balance
    bytes) -- each SDMA engine round-robins between rings at packet
    granularity, so the extra ring measurably lifts aggregate HBM
    throughput (~250 -> ~315 GB/s).  Stores go out in two column segments
    so segment A streams while the tail chunks still compute.

  * PSUM fp32 (3 pools: 3+2+3 banks), outputs cast to bf16 on the
    PSUM->SBUF copy; m1/m2 casts on DVE, m3 on ACT to split the copy load.
"""

import os
from contextlib import ExitStack

import ml_dtypes
import numpy as np

import concourse.bacc as bacc
import concourse.bass as bass
import concourse.mybir as mybir
import concourse.tile as tile
from concourse.bass_utils import run_bass_kernel_spmd

F32 = mybir.dt.float32
BF16 = mybir.dt.bfloat16
NPBF16 = ml_dtypes.bfloat16

B, D, M, S = 64, 8, 4, 50000
N_CORES = 8
BC = B // N_CORES  # batches per core
NG = 16            # sample groups packed into the 128 partitions
MW = 512           # matmul free-dim / PSUM bank width


def _geom(s):
    assert s % NG == 0, s
    gw = s // NG   # samples per group
    xc = gw + 2    # x cols: 1 past halo + gw + 1 fut halo (cols 0..gw+1 used)
    xc += xc % 2   # pad to even row bytes
    yc = gw + (gw % 2)
    return gw, xc, yc


def _build_nc(bc=BC, s=S):
    variant = os.environ.get("KERNEL_VARIANT", "full")  # perf bisection only
    gw, xc, yc = _geom(s)

    nc = bacc.Bacc(trn_type="TRN2")
    xp = nc.dram_tensor("xp", [bc, 128, xc], BF16, kind="ExternalInput")
    yp = nc.dram_tensor("yp", [bc, 64, yc], BF16, kind="ExternalInput")
    w = nc.dram_tensor("w_all", [128, 7 * 128], BF16, kind="ExternalInput")
    m_all = nc.dram_tensor("m_all", [bc, D, 3, s], BF16, kind="ExternalOutput")

    with tile.TileContext(nc) as tc, ExitStack() as ctx:
        singles = ctx.enter_context(tc.tile_pool(name="singles", bufs=1))
        xpool = ctx.enter_context(tc.tile_pool(name="xp", bufs=3))
        ypool = ctx.enter_context(tc.tile_pool(name="yp", bufs=3))
        o1p = ctx.enter_context(tc.tile_pool(name="o1", bufs=4))
        o2p = ctx.enter_context(tc.tile_pool(name="o2", bufs=4))
        o3p = ctx.enter_context(tc.tile_pool(name="o3", bufs=4))
        pp1 = ctx.enter_context(tc.tile_pool(name="pp1", bufs=3, space="PSUM"))
        pp2 = ctx.enter_context(tc.tile_pool(name="pp2", bufs=2, space="PSUM"))
        pp3 = ctx.enter_context(tc.tile_pool(name="pp3", bufs=3, space="PSUM"))

        w_sb = singles.tile([128, 7 * 128], BF16, tag="w")
        nc.sync.dma_start(out=w_sb[:], in_=w[:, :])

        wA1 = w_sb[:, 0:128]
        wB1 = w_sb[:, 128:256]
        wW2 = w_sb[:, 256:384]
        wA2 = w_sb[:, 384:512]
        wB2 = w_sb[:, 512:640]
        wA3 = w_sb[:, 640:768]
        wC3 = w_sb[0:64, 768:896]

        for b in range(bc):
            xoff = b * 128 * xc
            yoff = b * 64 * yc
            ooff = b * D * 3 * s

            x_t = xpool.tile([128, xc], BF16, tag="x")
            nc.scalar.dma_start(out=x_t[:], in_=bass.AP(xp, xoff, [[xc, 128], [1, xc]]))
            y_t = ypool.tile([64, yc], BF16, tag="y")
            nc.scalar.dma_start(out=y_t[:], in_=bass.AP(yp, yoff, [[yc, 64], [1, yc]]))
            if variant == "loads":
                continue

            o1 = o1p.tile([128, gw], BF16, tag="o1", name=f"o1_{b}")
            o2 = o2p.tile([128, gw], BF16, tag="o2", name=f"o2_{b}")
            o3 = o3p.tile([128, gw], BF16, tag="o3", name=f"o3_{b}")

            for h0 in range(0, gw, MW):
                hw = min(MW, gw - h0)
                cur = x_t[:, 1 + h0 : 1 + h0 + hw]
                past = x_t[:, h0 : h0 + hw]
                fut = x_t[:, 2 + h0 : 2 + h0 + hw]
                p1 = pp1.tile([128, MW], F32, tag="p1", name=f"p1_{b}_{h0}")
                nc.tensor.matmul(p1[:, 0:hw], wA1, cur, start=True, stop=False)
                nc.tensor.matmul(p1[:, 0:hw], wB1, past, start=False, stop=True)
                nc.vector.tensor_copy(out=o1[:, h0 : h0 + hw], in_=p1[:, 0:hw])
                p3 = pp3.tile([128, MW], F32, tag="p3", name=f"p3_{b}_{h0}")
                nc.tensor.matmul(p3[:, 0:hw], wA3, cur, start=True, stop=False)
                nc.tensor.matmul(
                    p3[:, 0:hw], wC3, y_t[:, h0 : h0 + hw], start=False, stop=True
                )
                nc.scalar.copy(out=o3[:, h0 : h0 + hw], in_=p3[:, 0:hw])
                p2 = pp2.tile([128, MW], F32, tag="p2", name=f"p2_{b}_{h0}")
                nc.tensor.matmul(p2[:, 0:hw], wA2, cur, start=True, stop=False)
                nc.tensor.matmul(p2[:, 0:hw], wB2, fut, start=False, stop=True)
                nc.vector.tensor_copy(out=o2[:, h0 : h0 + hw], in_=p2[:, 0:hw])

            if variant == "nostores":
                continue
            # stores split across the sync (HWDGE) and gpsimd (SWDGE) rings:
            # each SDMA engine round-robins between rings at packet
            # granularity, so a third stream hides more HBM latency.  o3
            # alternates so both store rings carry ~equal bytes; two column
            # segments let segment A stream while tail chunks compute.
            st_split = min(4 * MW, gw)
            for o_idx, o_t in ((0, o1), (1, o2), (2, o3)):
                if o_idx == 0:
                    eng = nc.sync
                elif o_idx == 1:
                    eng = nc.gpsimd
                else:
                    eng = nc.sync if b % 2 else nc.gpsimd
                eng.dma_start(
                    out=bass.AP(
                        m_all, ooff + o_idx * s, [[gw, NG], [3 * s, D], [1, st_split]]
                    ),
                    in_=o_t[:, 0:st_split],
                )
                if st_split < gw:
                    eng.dma_start(
                        out=bass.AP(
                            m_all,
                            ooff + o_idx * s + st_split,
                            [[gw, NG], [3 * s, D], [1, gw - st_split]],
                        ),
                        in_=o_t[:, st_split:gw],
                    )
    nc.finalize()
    return nc


def _build_weights(F, H, Q, R):
    """Host-side precompute (init-time work in the torch module)."""
    F64 = np.asarray(F, np.float64)
    H64 = np.asarray(H, np.float64)
    Qinv = np.linalg.inv(np.asarray(Q, np.float64))
    Rinv = np.linalg.inv(np.asarray(R, np.float64))
    A1 = -Qinv
    B1 = Qinv @ F64
    W2 = -F64.T
    C3 = H64.T @ Rinv          # (D, M)
    A3 = -(C3 @ H64)

    A2 = -(F64.T @ Qinv @ F64)
    B2 = F64.T @ Qinv

    eye = np.eye(NG)
    w = np.zeros((128, 7 * 128), NPBF16)
    for i, A in enumerate([A1, B1, W2, A2, B2, A3]):
        # lhsT[8g+j, 8g+i] = A[i, j]  ->  block-diag of A.T
        w[:, i * 128 : (i + 1) * 128] = np.kron(eye, A.T).astype(NPBF16)
    w[0:64, 768:896] = np.kron(eye, C3.T).astype(NPBF16)  # [4g+m, 8g+i] = C3[i, m]
    return w


def _pack_inputs(xs, ys, s):
    """xs (nb, D, s), ys (nb, s, M) f32 -> device images (bf16).

    xp[b, 8g+j, c] = xs[b, j, clip(g*gw + c - 1)]   (c in [0, xc))
    yp[b, 4g+m, c] = ys[b, clip(g*gw + c), m]       (c in [0, yc))
    """
    gw, xc, yc = _geom(s)
    nb = xs.shape[0]
    xs_bf = np.asarray(xs, np.float32).astype(NPBF16)
    g = np.arange(NG)[:, None] * gw
    xcols = np.clip(g + np.arange(xc)[None, :] - 1, 0, s - 1)  # (NG, xc)
    xp = xs_bf[:, :, xcols]                      # (nb, D, NG, xc)
    xp = np.ascontiguousarray(np.swapaxes(xp, 1, 2)).reshape(nb, 128, xc)

    ys_bf = np.swapaxes(np.asarray(ys, np.float32).astype(NPBF16), 1, 2)  # (nb, M, s)
    ycols = np.clip(g + np.arange(yc)[None, :], 0, s - 1)      # (NG, yc)
    yp = ys_bf[:, :, ycols]                      # (nb, M, NG, yc)
    yp = np.ascontiguousarray(np.swapaxes(yp, 1, 2)).reshape(nb, 64, yc)
    return xp, yp


_CACHE = {}


def _get_nc(bc=BC, s=S):
    key = (bc, s)
    if key not in _CACHE:
        _CACHE[key] = _build_nc(bc, s)
    return _CACHE[key]


def run(xs, ys, F, H, Q, R, trace=False, bc=BC, s=S):
    """Shard across 8 cores, run, gather.  Returns ((m1, m2, m3), results)."""
    nb = xs.shape[0]
    assert nb == bc * N_CORES and xs.shape[1:] == (D, s), xs.shape
    assert ys.shape == (nb, s, M), ys.shape
    xp, yp = _pack_inputs(xs, ys, s)
    w_all = _build_weights(F, H, Q, R)

    nc = _get_nc(bc, s)
    in_maps = [
        {
            "xp": np.ascontiguousarray(xp[i * bc : (i + 1) * bc]),
            "yp": np.ascontiguousarray(yp[i * bc : (i + 1) * bc]),
            "w_all": w_all,
        }
        for i in range(N_CORES)
    ]
    res = run_bass_kernel_spmd(nc, in_maps, core_ids=list(range(N_CORES)), trace=trace)
    m_full = np.concatenate([r["m_all"] for r in res.results], axis=0)  # (B,D,3,s)
    outs = tuple(
        np.ascontiguousarray(m_full[:, :, i, :]).astype(np.float32) for i in range(3)
    )
    return outs, res


def kernel(xs, ys, F, H, Q, R):
    trace = bool(int(os.environ.get("KERNEL_TRACE", "0")))
    outs, _ = run(xs, ys, F, H, Q, R, trace=trace)
    return outs
